# revision 11
# baseline (speedup 1.0000x reference)
"""Bass/Trainium2 kernel for nn_MultiHeadAttentionBlock_23502061043960.

Reference math (note: the module multiplies RAW scores with value — no
softmax in the output path — so the whole block is linear):

    out = (concat_h Q_h (K_h^T V_h) / 8) @ w_o.T + b_o
        where Q = q w_q^T, K = k w_k^T, V = v w_v^T   (biases are zero)

Linearity lets us contract the sequence dim first and never materialize
the [B,H,S,S] score tensor:

    A_b    = k_b^T v_b                     [512, 512]   (per batch)
    M_h    = w_k[h] A_b w_v[h]^T / 8       [64, 64]     (per head)
    W2     = w_o blockdiag(M_h^T)          [512, 512]
    Wfold  = w_q^T W2^T                    [512, 512]
    out_b  = q_b Wfold + b_o               (one dense matmul per row)

Sharding over 8 cores: core c owns batch c//4 and sequence-quarter c%4
of the output rows. Each core computes the full A_b from the full
k_b/v_b (4x redundant but collective-free: NRT collectives on this
stack have a ~20us latency floor — ncfw control-plane ~10us/ring-step —
which exceeds the 20.7us of PE redundancy they would remove), folds it
down to Wfold, and applies it to its own q rows.

Layout/staging choices (all host-side, free at HW time):
 - q is staged as q^T and the output returned as out^T, because the PE
   array contracts over the partition dim;
 - weights are staged transposed so they can be the stationary operand;
 - k/v are staged "pair-packed" ([128, 1024] tiles: two row-chunks
   side by side) so each DMA partition row is a 2 KiB contiguous run;
   weights are quad-packed the same way;
 - the 1/sqrt(dk) = 1/8 score scale is folded into the staged w_k.

Schedule optimizations over the v1 kernel:
 - DMA issue alternates between the two HWDGE rings (nc.sync = SP and
   nc.scalar = ACT) so descriptor generation for the k/v stream is not
   serialized on one FIFO ring, and the first k/v chunk's k and v
   tiles load in parallel on the two rings;
 - the A and Y PSUM->SBUF casts alternate vector/scalar engines (the
   scalar engine's ACT table is pre-warmed by a tiny copy at kernel
   start so the lazy 1.3us ACT_TABLE_LOAD doesn't sit on the fold's
   critical path);
 - blockdiag(M^T) tiles are built by two direct [64,64] PSUM->SBUF
   engine copies into a pre-zeroed tile (the diag blocks of the G band
   PSUM tiles are exactly M_h^T), replacing the serialized
   scalar-gather + SBUF->SBUF DMA quadrant writes of v1 (~4-5us of PE
   gaps in the fold; fold+apply window 25.2us -> 19.8us measured);
 - output tiles are paired into [128, 1024] stores (2KB DRAM runs) and
   the bias-adds alternate vector/scalar.

Schedule experiments that did NOT help (measured): loading the first
chunk as [128,128] column slices consumed by N=128 matmuls starts the
PE ~0.8us earlier but loses ~4us net — the early fine-grained matmuls
stall on descriptor-trickle DMA and run during the PE pstate ramp.
Beware: matmul start=True zeroes the WHOLE PSUM bank, not just the
addressed sub-region — sub-region accumulation needs start=True only
on the first write into the tile.

dtype: all matmul inputs fp16 (host-staged; same 2-byte DMA cost and
full 1-cycle/row PE rate as bf16 but 10 mantissa bits), fp32 PSUM
accumulation throughout, fp16 output upcast on host. Measured rel err
vs the fp32 reference is ~7e-4, ~28x inside the 2e-2 gate. fp8 paths
were evaluated and rejected: e4m3 k/v gives 3.1e-2 rel err (gate is
2e-2), and DoubleRow is only ~1.44x so a hi/lo-split 3-pass A would be
slower than one fp16 pass.
"""

import ml_dtypes
import numpy as np

import concourse.mybir as mybir
import concourse.tile as tile
from concourse import bacc
from concourse.bass_utils import run_bass_kernel_spmd

B = 2
S = 4096
D = 512
H = 8
DK = 64
N_CORES = 8
SQ = S // 4  # 1024 output rows per core
P = 128
F32 = mybir.dt.float32

USE_BF16 = True

_compiled = {}

LAST_RESULTS = None  # test harness reads exec_time_ns / trace from here
RUN_KW = {}  # test harness can inject trace kwargs


def _build():
    nc = bacc.Bacc()

    DT = mybir.dt.float16 if USE_BF16 else mybir.dt.float32r

    # k/v pair-packed: 2 row-chunks of [128, 512] side by side in one
    # [128, 1024] tile -> every DMA partition row is a 2 KiB run.
    kb = nc.declare_dram_parameter("kb", [S // 2, 2 * D], DT, isOutput=False)
    vb = nc.declare_dram_parameter("vb", [S // 2, 2 * D], DT, isOutput=False)
    qT = nc.declare_dram_parameter("qT", [D, SQ], DT, isOutput=False)
    wkT = nc.declare_dram_parameter("wkT", [P, 4 * D], DT, isOutput=False)
    wvT = nc.declare_dram_parameter("wvT", [P, 4 * D], DT, isOutput=False)
    wq = nc.declare_dram_parameter("wq", [P, 4 * D], DT, isOutput=False)
    woT = nc.declare_dram_parameter("woT", [P, 4 * D], DT, isOutput=False)
    bo = nc.declare_dram_parameter("bo", [P, 4], F32, isOutput=False)
    outT = nc.declare_dram_parameter("outT", [D, SQ], DT, isOutput=True)

    kb_v = kb.rearrange("(n p) d -> n p d", p=P)  # 16 x [128, 1024]
    vb_v = vb.rearrange("(n p) d -> n p d", p=P)
    qT_v = qT.rearrange("(n p) d -> n p d", p=P)  # 4 x [128, 1024]
    outT_v = outT.rearrange("(n p) d -> n p d", p=P)  # 4 x [128, 1024]

    NKC = S // P  # 32 contraction chunks for A
    NDC = D // P  # 4 chunks of the model dim
    NG = NKC // 2  # 16 pair-packed k/v tiles

    # Alternate the two HWDGE rings (SP / ACT sequencers) for loads.
    def ring(i):
        return nc.sync if i % 2 == 0 else nc.scalar

    with tile.TileContext(nc) as tc:
        with (
            tc.tile_pool(name="w", bufs=1) as wp,
            tc.tile_pool(name="kv", bufs=1) as kvp,
            tc.tile_pool(name="qt", bufs=1) as qtp,
            tc.tile_pool(name="work", bufs=NDC) as wkpool,
            tc.tile_pool(name="small", bufs=1) as smallp,
            tc.tile_pool(name="ot", bufs=4) as otp,
            tc.tile_pool(name="psB", bufs=4, space="PSUM") as psb,
        ):
            # ACT-table prewarm: the first scalar-engine compute op lazily
            # loads its activation table (~1.3us); trigger it early so
            # fold-phase scalar casts start instantly.
            warm = smallp.tile([P, 4], F32, name="warm", tag="warm")
            bo_t = wp.tile([P, 4], F32, name="bo", tag="bo")

            # bd tiles memset early (gpsimd) so only two diag-block copies
            # remain on the fold's critical path.
            bd_t = [smallp.tile([P, P], DT, name=f"bd{p}", tag=f"bd{p}") for p in range(NDC)]
            for p in range(NDC):
                nc.gpsimd.memset(bd_t[p][:].bitcast(mybir.dt.uint32), 0)

            a_sb = []
            with tc.tile_pool(name="psA", bufs=NDC, space="PSUM") as psa:
                # ---- phase 1: A = k^T v, streaming k/v chunk pairs -------
                a_ps = [psa.tile([P, D], F32, name=f"aps{m}", tag="aps") for m in range(NDC)]

                # first pair as 2 standalone chunk tiles on alternating
                # rings (k on SP, v on ACT arrive in parallel) so the very
                # first matmul only waits on two 0.25 MiB transfers
                k0 = [kvp.tile([P, D], DT, name=f"k0{j}", tag=f"k0{j}") for j in range(2)]
                v0 = [kvp.tile([P, D], DT, name=f"v0{j}", tag=f"v0{j}") for j in range(2)]
                for j in range(2):
                    js = slice(j * D, (j + 1) * D)
                    nc.sync.dma_start(out=k0[j][:], in_=kb_v[0][:, js])
                    nc.scalar.dma_start(out=v0[j][:], in_=vb_v[0][:, js])
                for j in range(2):
                    for m in range(NDC):
                        nc.tensor.matmul(
                            a_ps[m][:],
                            k0[j][:, m * P : (m + 1) * P],
                            v0[j][:],
                            start=(j == 0),
                            stop=False,
                        )

                # remaining pair tiles, alternating rings
                k_t = [kvp.tile([P, 2 * D], DT, name=f"k{i}", tag=f"k{i}") for i in range(1, NG)]
                v_t = [kvp.tile([P, 2 * D], DT, name=f"v{i}", tag=f"v{i}") for i in range(1, NG)]
                for g in range(1, NG):
                    ring(g).dma_start(out=k_t[g - 1][:], in_=kb_v[g])
                    ring(g + 1).dma_start(out=v_t[g - 1][:], in_=vb_v[g])
                    for j in range(2):
                        for m in range(NDC):
                            nc.tensor.matmul(
                                a_ps[m][:],
                                k_t[g - 1][:, j * D + m * P : j * D + (m + 1) * P],
                                v_t[g - 1][:, j * D : (j + 1) * D],
                                start=False,
                                stop=(g == NG - 1 and j == 1),
                            )

                # ---- remaining loads issued behind the k/v stream --------
                wk_h = [wp.tile([P, 2 * D], DT, name=f"wkh{i}", tag=f"wkh{i}") for i in range(2)]
                wv_h = [wp.tile([P, 2 * D], DT, name=f"wvh{i}", tag=f"wvh{i}") for i in range(2)]
                wq_t = wp.tile([P, 4 * D], DT, name="wqt", tag="wqt")
                wo_t = wp.tile([P, 4 * D], DT, name="wot", tag="wot")
                for i in range(2):
                    hs = slice(i * 2 * D, (i + 1) * 2 * D)
                    ring(i).dma_start(out=wk_h[i][:], in_=wkT[:, hs])
                    ring(i + 1).dma_start(out=wv_h[i][:], in_=wvT[:, hs])
                qt_t = [qtp.tile([P, SQ], DT, name=f"q{i}", tag=f"q{i}") for i in range(NDC)]
                for i in range(NDC):
                    ring(i).dma_start(out=qt_t[i][:], in_=qT_v[i])
                nc.sync.dma_start(out=wo_t[:], in_=woT[:])
                nc.scalar.dma_start(out=wq_t[:], in_=wq[:])
                nc.sync.dma_start(out=bo_t[:], in_=bo[:])
                nc.scalar.copy(warm[:], bo_t[:])  # ACT-table prewarm

                # A casts split across vector+scalar so the fold starts
                # ~1.3us after the last A matmul instead of ~2.5us.
                for m in range(NDC):
                    t = wkpool.tile([P, D], DT, name="a", tag="a")
                    if m % 2 == 0:
                        nc.vector.tensor_copy(t[:], a_ps[m][:])
                    else:
                        nc.scalar.copy(t[:], a_ps[m][:])
                    a_sb.append(t)

            with tc.tile_pool(name="psW", bufs=4, space="PSUM") as psw:
                # ---- fold F1+F2, chunk-pipelined: F2 (band of G = w_v Y^T,
                # whose diag blocks are M_h^T) accumulates over vd chunks, so
                # each F1 output chunk feeds F2 as soon as it is copied.
                g_ps = [psw.tile([P, P], F32, name=f"gps{m}", tag="pw") for m in range(NDC)]
                for kc in range(NDC):
                    y_ps = psb.tile([P, D], F32, name="yps", tag="ps")
                    for kd in range(NDC):
                        nc.tensor.matmul(
                            y_ps[:],
                            a_sb[kd][:, kc * P : (kc + 1) * P],
                            wk_h[kd // 2][:, (kd % 2) * D : (kd % 2 + 1) * D],
                            start=(kd == 0),
                            stop=(kd == NDC - 1),
                        )
                    yT = wkpool.tile([P, D], DT, name="yT", tag="yT")
                    if kc % 2 == 0:
                        nc.vector.tensor_copy(yT[:], y_ps[:])
                    else:
                        nc.scalar.copy(yT[:], y_ps[:])
                    for mp in range(NDC):
                        nc.tensor.matmul(
                            g_ps[mp][:],
                            wv_h[kc // 2][:, (kc % 2) * D + mp * P : (kc % 2) * D + (mp + 1) * P],
                            yT[:, mp * P : (mp + 1) * P],
                            start=(kc == 0),
                            stop=(kc == NDC - 1),
                        )

                # ---- phase 2b: W2^T = BD(M) woT  (W2 = w_o BD(M)^T) ------
                # The diag [64,64] blocks of g_ps[p] are M_2p^T / M_2p+1^T;
                # copy them straight into the pre-zeroed bd tiles.
                w2_sb = []
                for p in range(NDC):
                    nc.vector.tensor_copy(bd_t[p][0:DK, 0:DK], g_ps[p][0:DK, 0:DK])
                    nc.scalar.copy(bd_t[p][DK:P, DK:P], g_ps[p][DK:P, DK:P])
                    w2_ps = psb.tile([P, D], F32, name="w2ps", tag="ps")
                    nc.tensor.matmul(w2_ps[:], bd_t[p][:], wo_t[:, p * D : (p + 1) * D], start=True, stop=True)
                    t = wkpool.tile([P, D], DT, name="w2", tag="w2")
                    if p % 2 == 0:
                        nc.vector.tensor_copy(t[:], w2_ps[:])
                    else:
                        nc.scalar.copy(t[:], w2_ps[:])
                    w2_sb.append(t)

                # ---- fold Wfold = w_q^T W2^T  (out = q Wfold + b_o) ------
                wf_sb = []
                for m in range(NDC):
                    wf_ps = psb.tile([P, D], F32, name="wfps", tag="ps")
                    for kc in range(NDC):
                        nc.tensor.matmul(
                            wf_ps[:],
                            wq_t[:, kc * D + m * P : kc * D + (m + 1) * P],
                            w2_sb[kc][:],
                            start=(kc == 0),
                            stop=(kc == NDC - 1),
                        )
                    t = wkpool.tile([P, D], DT, name="wf", tag="wf")
                    if m % 2 == 0:
                        nc.vector.tensor_copy(t[:], wf_ps[:])
                    else:
                        nc.scalar.copy(t[:], wf_ps[:])
                    wf_sb.append(t)

                # ---- phase 2c: out^T = Wfold^T Qp^T + b_o ----------------
                # Pair the two 512-col halves into one [128,1024] store so
                # DRAM runs are 2KB; bias adds alternate vector/scalar.
                for m in range(NDC):
                    o_sb = otp.tile([P, SQ], DT, name="osb", tag="osb")
                    for nn in range(SQ // D):
                        ns = slice(nn * D, (nn + 1) * D)
                        o_ps = psw.tile([P, D], F32, name="ops", tag="pw")
                        for kc in range(NDC):
                            nc.tensor.matmul(
                                o_ps[:],
                                wf_sb[kc][:, m * P : (m + 1) * P],
                                qt_t[kc][:, ns],
                                start=(kc == 0),
                                stop=(kc == NDC - 1),
                            )
                        if (2 * m + nn) % 2 == 0:
                            nc.vector.tensor_scalar_add(o_sb[:, ns], o_ps[:], bo_t[:, m : m + 1])
                        else:
                            nc.scalar.add(o_sb[:, ns], o_ps[:], bo_t[:, m : m + 1])
                    ring(m).dma_start(out=outT_v[m][:], in_=o_sb[:])

    nc.compile()
    return nc


def kernel(q, k, v, w_q, b_q, w_k, b_k, w_v, b_v, w_o, b_o):
    global LAST_RESULTS
    key = ("nc", USE_BF16)
    if key not in _compiled:
        _compiled[key] = _build()
    nc = _compiled[key]

    np_dt = np.float16 if USE_BF16 else np.float32

    def packn(x, w):  # [N, 512] -> [N//w, w*512]: w row-chunks side by side
        n = x.shape[0] // (w * P)
        return np.ascontiguousarray(
            x.reshape(n, w, P, D).transpose(0, 2, 1, 3).reshape(n * P, w * D)
        )

    def pack4(x):
        return packn(x, 4)

    q = np.asarray(q, dtype=np.float32)
    kc_ = [packn(np.asarray(k[b], np.float32).astype(np_dt), 2) for b in range(B)]
    vc_ = [packn(np.asarray(v[b], np.float32).astype(np_dt), 2) for b in range(B)]
    wkT = pack4((np.asarray(w_k, np.float32).T * 0.125).astype(np_dt))
    wvT = pack4(np.asarray(w_v, np.float32).T.astype(np_dt))
    wqn = pack4(np.asarray(w_q, np.float32).astype(np_dt))
    woT = pack4(np.asarray(w_o, np.float32).T.astype(np_dt))
    bo = np.ascontiguousarray(np.asarray(b_o, np.float32).reshape(4, P).T)

    in_maps = []
    for c in range(N_CORES):
        b, quarter = divmod(c, 4)
        rows = slice(quarter * SQ, (quarter + 1) * SQ)
        in_maps.append(
            {
                "kb": kc_[b],
                "vb": vc_[b],
                "qT": np.ascontiguousarray(q[b, rows, :].T).astype(np_dt),
                "wkT": wkT,
                "wvT": wvT,
                "wq": wqn,
                "woT": woT,
                "bo": bo,
            }
        )

    res = run_bass_kernel_spmd(nc, in_maps, list(range(N_CORES)), **RUN_KW)
    LAST_RESULTS = res

    out = np.empty((B, S, D), dtype=np.float32)
    for c in range(N_CORES):
        b, quarter = divmod(c, 4)
        rows = slice(quarter * SQ, (quarter + 1) * SQ)
        out[b, rows, :] = res.results[c]["outT"].T.astype(np.float32)
    return out


# revision 12
# speedup vs baseline: 1.0841x; 1.0841x over previous
"""Bass/Trainium2 kernel for nn_MultiHeadAttentionBlock_23502061043960.

Reference math (note: the module multiplies RAW scores with value — no
softmax in the output path — so the whole block is linear):

    out = (concat_h Q_h (K_h^T V_h) / 8) @ w_o.T + b_o
        where Q = q w_q^T, K = k w_k^T, V = v w_v^T   (biases are zero)

Linearity lets us contract the sequence dim first and never materialize
the [B,H,S,S] score tensor:

    A_b    = k_b^T v_b                     [512, 512]   (per batch)
    M_h    = w_k[h] A_b w_v[h]^T / 8       [64, 64]     (per head)
    W2     = w_o blockdiag(M_h^T)          [512, 512]
    Wfold  = w_q^T W2^T                    [512, 512]
    out_b  = q_b Wfold + b_o               (one dense matmul per row)

Sharding over 8 cores: core c owns batch c//4 and sequence-quarter c%4
of the output rows. Each core computes the full A_b from the full
k_b/v_b (4x redundant but collective-free: NRT collectives on this
stack have a ~20us latency floor — ncfw control-plane ~10us/ring-step —
which exceeds the 20.7us of PE redundancy they would remove), folds it
down to Wfold, and applies it to its own q rows.

Layout/staging choices (all host-side, free at HW time):
 - q is staged as q^T and the output returned as out^T, because the PE
   array contracts over the partition dim;
 - weights are staged transposed so they can be the stationary operand;
 - k/v are staged "pair-packed" ([128, 1024] tiles: two row-chunks
   side by side) so each DMA partition row is a 2 KiB contiguous run;
   weights are quad-packed the same way;
 - the 1/sqrt(dk) = 1/8 score scale is folded into the staged w_k.

Schedule optimizations over the v1 kernel:
 - DMA issue alternates between the two HWDGE rings (nc.sync = SP and
   nc.scalar = ACT) so descriptor generation for the k/v stream is not
   serialized on one FIFO ring, and the first k/v chunk's k and v
   tiles load in parallel on the two rings;
 - the A and Y PSUM->SBUF casts alternate vector/scalar engines (the
   scalar engine's ACT table is pre-warmed by a tiny copy at kernel
   start so the lazy 1.3us ACT_TABLE_LOAD doesn't sit on the fold's
   critical path);
 - blockdiag(M^T) tiles are built by two direct [64,64] PSUM->SBUF
   engine copies into a pre-zeroed tile (the diag blocks of the G band
   PSUM tiles are exactly M_h^T), replacing the serialized
   scalar-gather + SBUF->SBUF DMA quadrant writes of v1 (~4-5us of PE
   gaps in the fold; fold+apply window 25.2us -> 19.8us measured);
 - output tiles are paired into [128, 1024] stores (2KB DRAM runs) and
   the bias-adds alternate vector/scalar.

Schedule experiments that did NOT help (measured): loading the first
chunk as [128,128] column slices consumed by N=128 matmuls starts the
PE ~0.8us earlier but loses ~4us net — the early fine-grained matmuls
stall on descriptor-trickle DMA and run during the PE pstate ramp.
Beware: matmul start=True zeroes the WHOLE PSUM bank, not just the
addressed sub-region — sub-region accumulation needs start=True only
on the first write into the tile.

dtype: all matmul inputs fp16 (host-staged; same 2-byte DMA cost and
full 1-cycle/row PE rate as bf16 but 10 mantissa bits), fp32 PSUM
accumulation throughout, fp16 output upcast on host. Measured rel err
vs the fp32 reference is ~7e-4, ~28x inside the 2e-2 gate. fp8 paths
were evaluated and rejected: e4m3 k/v gives 3.1e-2 rel err (gate is
2e-2), and DoubleRow is only ~1.44x so a hi/lo-split 3-pass A would be
slower than one fp16 pass.
"""

import ml_dtypes
import numpy as np

import concourse.mybir as mybir
import concourse.tile as tile
from concourse import bacc
from concourse.bass_utils import run_bass_kernel_spmd

B = 2
S = 4096
D = 512
H = 8
DK = 64
N_CORES = 8
SQ = S // 4  # 1024 output rows per core
P = 128
F32 = mybir.dt.float32

USE_BF16 = True

_compiled = {}

LAST_RESULTS = None  # test harness reads exec_time_ns / trace from here
RUN_KW = {}  # test harness can inject trace kwargs


def _build():
    nc = bacc.Bacc()

    DT = mybir.dt.float16 if USE_BF16 else mybir.dt.float32r

    # k/v pair-packed: 2 row-chunks of [128, 512] side by side in one
    # [128, 1024] tile -> every DMA partition row is a 2 KiB run.
    kb = nc.declare_dram_parameter("kb", [S // 2, 2 * D], DT, isOutput=False)
    vb = nc.declare_dram_parameter("vb", [S // 2, 2 * D], DT, isOutput=False)
    qT = nc.declare_dram_parameter("qT", [D, SQ], DT, isOutput=False)
    wkT = nc.declare_dram_parameter("wkT", [P, 4 * D], DT, isOutput=False)
    wvT = nc.declare_dram_parameter("wvT", [P, 4 * D], DT, isOutput=False)
    wq = nc.declare_dram_parameter("wq", [P, 4 * D], DT, isOutput=False)
    woT = nc.declare_dram_parameter("woT", [P, 4 * D], DT, isOutput=False)
    bo = nc.declare_dram_parameter("bo", [P, 4], F32, isOutput=False)
    outT = nc.declare_dram_parameter("outT", [D, SQ], DT, isOutput=True)

    kb_v = kb.rearrange("(n p) d -> n p d", p=P)  # 16 x [128, 1024]
    vb_v = vb.rearrange("(n p) d -> n p d", p=P)
    qT_v = qT.rearrange("(n p) d -> n p d", p=P)  # 4 x [128, 1024]
    outT_v = outT.rearrange("(n p) d -> n p d", p=P)  # 4 x [128, 1024]

    NKC = S // P  # 32 contraction chunks for A
    NDC = D // P  # 4 chunks of the model dim
    NG = NKC // 2  # 16 pair-packed k/v tiles

    # Alternate the two HWDGE rings (SP / ACT sequencers) for loads.
    def ring(i):
        return nc.sync if i % 2 == 0 else nc.scalar

    with tile.TileContext(nc) as tc:
        with (
            tc.tile_pool(name="w", bufs=1) as wp,
            tc.tile_pool(name="kv", bufs=1) as kvp,
            tc.tile_pool(name="qt", bufs=1) as qtp,
            tc.tile_pool(name="work", bufs=NDC) as wkpool,
            tc.tile_pool(name="small", bufs=1) as smallp,
            tc.tile_pool(name="ot", bufs=4) as otp,
            tc.tile_pool(name="psB", bufs=4, space="PSUM") as psb,
        ):
            # ACT-table prewarm: the first scalar-engine compute op lazily
            # loads its activation table (~1.3us); trigger it early so
            # fold-phase scalar casts start instantly.
            warm = smallp.tile([P, 4], F32, name="warm", tag="warm")
            bo_t = wp.tile([P, 4], F32, name="bo", tag="bo")

            # bd tiles memset early (gpsimd) so only two diag-block copies
            # remain on the fold's critical path.
            bd_t = [smallp.tile([P, P], DT, name=f"bd{p}", tag=f"bd{p}") for p in range(NDC)]
            for p in range(NDC):
                nc.gpsimd.memset(bd_t[p][:].bitcast(mybir.dt.uint32), 0)

            a_sb = []
            with tc.tile_pool(name="psA", bufs=NDC, space="PSUM") as psa:
                # ---- phase 1: A = k^T v, streaming k/v chunk pairs -------
                a_ps = [psa.tile([P, D], F32, name=f"aps{m}", tag="aps") for m in range(NDC)]

                # first pair as 2 standalone chunk tiles on alternating
                # rings (k on SP, v on ACT arrive in parallel) so the very
                # first matmul only waits on two 0.25 MiB transfers
                k0 = [kvp.tile([P, D], DT, name=f"k0{j}", tag=f"k0{j}") for j in range(2)]
                v0 = [kvp.tile([P, D], DT, name=f"v0{j}", tag=f"v0{j}") for j in range(2)]
                for j in range(2):
                    js = slice(j * D, (j + 1) * D)
                    nc.sync.dma_start(out=k0[j][:], in_=kb_v[0][:, js])
                    nc.scalar.dma_start(out=v0[j][:], in_=vb_v[0][:, js])
                for j in range(2):
                    for m in range(NDC):
                        nc.tensor.matmul(
                            a_ps[m][:],
                            k0[j][:, m * P : (m + 1) * P],
                            v0[j][:],
                            start=(j == 0),
                            stop=False,
                        )

                # remaining pair tiles, alternating rings; the fold-phase
                # weight loads are interleaved mid-stream so they land
                # before phase 1 drains (issued only behind the k/v stream
                # they arrive ~6us after the last A matmul and stall the
                # fold's first Y matmuls — measured 5.5us PE gap).
                wk_h = [wp.tile([P, 2 * D], DT, name=f"wkh{i}", tag=f"wkh{i}") for i in range(2)]
                wv_h = [wp.tile([P, 2 * D], DT, name=f"wvh{i}", tag=f"wvh{i}") for i in range(2)]
                wq_t = wp.tile([P, 4 * D], DT, name="wqt", tag="wqt")
                wo_t = wp.tile([P, 4 * D], DT, name="wot", tag="wot")
                k_t = [kvp.tile([P, 2 * D], DT, name=f"k{i}", tag=f"k{i}") for i in range(1, NG)]
                v_t = [kvp.tile([P, 2 * D], DT, name=f"v{i}", tag=f"v{i}") for i in range(1, NG)]
                for g in range(1, NG):
                    ring(g).dma_start(out=k_t[g - 1][:], in_=kb_v[g])
                    ring(g + 1).dma_start(out=v_t[g - 1][:], in_=vb_v[g])
                    if g == 5 or g == 7:
                        i = (g - 5) // 2
                        hs = slice(i * 2 * D, (i + 1) * 2 * D)
                        ring(i).dma_start(out=wk_h[i][:], in_=wkT[:, hs])
                        ring(i + 1).dma_start(out=wv_h[i][:], in_=wvT[:, hs])
                    elif g == 9:
                        nc.sync.dma_start(out=wo_t[:], in_=woT[:])
                        nc.scalar.dma_start(out=wq_t[:], in_=wq[:])
                    for j in range(2):
                        for m in range(NDC):
                            nc.tensor.matmul(
                                a_ps[m][:],
                                k_t[g - 1][:, j * D + m * P : j * D + (m + 1) * P],
                                v_t[g - 1][:, j * D : (j + 1) * D],
                                start=False,
                                stop=(g == NG - 1 and j == 1),
                            )

                # q tiles are only needed by the apply (~15us after the
                # fold starts) — they stay behind the k/v stream.
                qt_t = [qtp.tile([P, SQ], DT, name=f"q{i}", tag=f"q{i}") for i in range(NDC)]
                for i in range(NDC):
                    ring(i).dma_start(out=qt_t[i][:], in_=qT_v[i])
                nc.sync.dma_start(out=bo_t[:], in_=bo[:])
                nc.scalar.copy(warm[:], bo_t[:])  # ACT-table prewarm

                # A casts split across vector+scalar so the fold starts
                # ~1.3us after the last A matmul instead of ~2.5us.
                for m in range(NDC):
                    t = wkpool.tile([P, D], DT, name="a", tag="a")
                    if m % 2 == 0:
                        nc.vector.tensor_copy(t[:], a_ps[m][:])
                    else:
                        nc.scalar.copy(t[:], a_ps[m][:])
                    a_sb.append(t)

            with tc.tile_pool(name="psW", bufs=4, space="PSUM") as psw:
                # ---- fold F1+F2, chunk-pipelined: F2 (band of G = w_v Y^T,
                # whose diag blocks are M_h^T) accumulates over vd chunks, so
                # each F1 output chunk feeds F2 as soon as it is copied.
                g_ps = [psw.tile([P, P], F32, name=f"gps{m}", tag="pw") for m in range(NDC)]
                for kc in range(NDC):
                    y_ps = psb.tile([P, D], F32, name="yps", tag="ps")
                    for kd in range(NDC):
                        nc.tensor.matmul(
                            y_ps[:],
                            a_sb[kd][:, kc * P : (kc + 1) * P],
                            wk_h[kd // 2][:, (kd % 2) * D : (kd % 2 + 1) * D],
                            start=(kd == 0),
                            stop=(kd == NDC - 1),
                        )
                    yT = wkpool.tile([P, D], DT, name="yT", tag="yT")
                    if kc % 2 == 0:
                        nc.vector.tensor_copy(yT[:], y_ps[:])
                    else:
                        nc.scalar.copy(yT[:], y_ps[:])
                    for mp in range(NDC):
                        nc.tensor.matmul(
                            g_ps[mp][:],
                            wv_h[kc // 2][:, (kc % 2) * D + mp * P : (kc % 2) * D + (mp + 1) * P],
                            yT[:, mp * P : (mp + 1) * P],
                            start=(kc == 0),
                            stop=(kc == NDC - 1),
                        )

                # ---- phase 2b: W2^T = BD(M) woT  (W2 = w_o BD(M)^T) ------
                # The diag [64,64] blocks of g_ps[p] are M_2p^T / M_2p+1^T;
                # copy them straight into the pre-zeroed bd tiles.
                w2_sb = []
                for p in range(NDC):
                    nc.vector.tensor_copy(bd_t[p][0:DK, 0:DK], g_ps[p][0:DK, 0:DK])
                    nc.scalar.copy(bd_t[p][DK:P, DK:P], g_ps[p][DK:P, DK:P])
                    w2_ps = psb.tile([P, D], F32, name="w2ps", tag="ps")
                    nc.tensor.matmul(w2_ps[:], bd_t[p][:], wo_t[:, p * D : (p + 1) * D], start=True, stop=True)
                    t = wkpool.tile([P, D], DT, name="w2", tag="w2")
                    if p % 2 == 0:
                        nc.vector.tensor_copy(t[:], w2_ps[:])
                    else:
                        nc.scalar.copy(t[:], w2_ps[:])
                    w2_sb.append(t)

                # ---- fold Wfold = w_q^T W2^T  (out = q Wfold + b_o) ------
                wf_sb = []
                for m in range(NDC):
                    wf_ps = psb.tile([P, D], F32, name="wfps", tag="ps")
                    for kc in range(NDC):
                        nc.tensor.matmul(
                            wf_ps[:],
                            wq_t[:, kc * D + m * P : kc * D + (m + 1) * P],
                            w2_sb[kc][:],
                            start=(kc == 0),
                            stop=(kc == NDC - 1),
                        )
                    t = wkpool.tile([P, D], DT, name="wf", tag="wf")
                    if m % 2 == 0:
                        nc.vector.tensor_copy(t[:], wf_ps[:])
                    else:
                        nc.scalar.copy(t[:], wf_ps[:])
                    wf_sb.append(t)

                # ---- phase 2c: out^T = Wfold^T Qp^T + b_o ----------------
                # Pair the two 512-col halves into one [128,1024] store so
                # DRAM runs are 2KB; bias adds alternate vector/scalar.
                for m in range(NDC):
                    o_sb = otp.tile([P, SQ], DT, name="osb", tag="osb")
                    for nn in range(SQ // D):
                        ns = slice(nn * D, (nn + 1) * D)
                        o_ps = psw.tile([P, D], F32, name="ops", tag="pw")
                        for kc in range(NDC):
                            nc.tensor.matmul(
                                o_ps[:],
                                wf_sb[kc][:, m * P : (m + 1) * P],
                                qt_t[kc][:, ns],
                                start=(kc == 0),
                                stop=(kc == NDC - 1),
                            )
                        if (2 * m + nn) % 2 == 0:
                            nc.vector.tensor_scalar_add(o_sb[:, ns], o_ps[:], bo_t[:, m : m + 1])
                        else:
                            nc.scalar.add(o_sb[:, ns], o_ps[:], bo_t[:, m : m + 1])
                    ring(m).dma_start(out=outT_v[m][:], in_=o_sb[:])

    nc.compile()
    return nc


def kernel(q, k, v, w_q, b_q, w_k, b_k, w_v, b_v, w_o, b_o):
    global LAST_RESULTS
    key = ("nc", USE_BF16)
    if key not in _compiled:
        _compiled[key] = _build()
    nc = _compiled[key]

    np_dt = np.float16 if USE_BF16 else np.float32

    def packn(x, w):  # [N, 512] -> [N//w, w*512]: w row-chunks side by side
        n = x.shape[0] // (w * P)
        return np.ascontiguousarray(
            x.reshape(n, w, P, D).transpose(0, 2, 1, 3).reshape(n * P, w * D)
        )

    def pack4(x):
        return packn(x, 4)

    q = np.asarray(q, dtype=np.float32)
    kc_ = [packn(np.asarray(k[b], np.float32).astype(np_dt), 2) for b in range(B)]
    vc_ = [packn(np.asarray(v[b], np.float32).astype(np_dt), 2) for b in range(B)]
    wkT = pack4((np.asarray(w_k, np.float32).T * 0.125).astype(np_dt))
    wvT = pack4(np.asarray(w_v, np.float32).T.astype(np_dt))
    wqn = pack4(np.asarray(w_q, np.float32).astype(np_dt))
    woT = pack4(np.asarray(w_o, np.float32).T.astype(np_dt))
    bo = np.ascontiguousarray(np.asarray(b_o, np.float32).reshape(4, P).T)

    in_maps = []
    for c in range(N_CORES):
        b, quarter = divmod(c, 4)
        rows = slice(quarter * SQ, (quarter + 1) * SQ)
        in_maps.append(
            {
                "kb": kc_[b],
                "vb": vc_[b],
                "qT": np.ascontiguousarray(q[b, rows, :].T).astype(np_dt),
                "wkT": wkT,
                "wvT": wvT,
                "wq": wqn,
                "woT": woT,
                "bo": bo,
            }
        )

    res = run_bass_kernel_spmd(nc, in_maps, list(range(N_CORES)), **RUN_KW)
    LAST_RESULTS = res

    out = np.empty((B, S, D), dtype=np.float32)
    for c in range(N_CORES):
        b, quarter = divmod(c, 4)
        rows = slice(quarter * SQ, (quarter + 1) * SQ)
        out[b, rows, :] = res.results[c]["outT"].T.astype(np.float32)
    return out
